# revision 8
# baseline (speedup 1.0000x reference)
"""BitNet MNIST MLP forward on 8 Trainium2 NeuronCores (pure data parallel).

Reference math (per _bitlinear): out = (x/sx) @ w_q.T * sx with per-row
sx = max(|x|) -- the activation scale cancels exactly, so we compute
x @ w_q.T directly.  Ternary w_q is precomputed on host (exact in bf16).

Per-core dataflow (batch shard 8192 rows, chunks of 512 batch columns):
  activations live feature-major [feat_part(128) x batch_free] in SBUF, so
  every layer's matmul contracts features on partitions with stationary
  (pre-transposed) weights and NO on-chip transposes.
  RMS mean(h^2) over the 1024 features = ones(1/1024)-matmul accumulated
  over the 8 feature tiles -> replicated [128, 512] PSUM value.
  rsqrt = int bit-trick seed + 2 Newton iterations on DVE (no ACT table
  thrash; ScalarE only runs {copy, square, gelu} = one table set).
  gelu(h*rinv*g): rinv via DVE tensor-tensor, g via per-partition ACT scale.
"""

import os
from contextlib import ExitStack

import numpy as np
import ml_dtypes

import concourse.bacc as bacc
import concourse.bass as bass
import concourse.mybir as mybir
import concourse.tile as tile
from concourse.bass_utils import run_bass_kernel_spmd

N_CORES = 8
B, IN, H, OUT = 65536, 784, 1024, 10
BPC = B // N_CORES  # 8192 rows per core
KP = 896            # 784 padded to 7*128
K1 = KP // 128      # 7 contraction tiles, layer 1
K2 = H // 128       # 8 contraction tiles, layers 2/3
HO = H // 128       # 8 output-feature tiles
BS = 512            # batch columns per chunk
NB = BPC // BS      # 16 chunks
EPS_Q = 1e-5
MAGIC = 0x5F3759DF

F32 = mybir.dt.float32
BF16 = mybir.dt.bfloat16
I32 = mybir.dt.int32
ALU = mybir.AluOpType
ACTF = mybir.ActivationFunctionType

_cache = {}
LAST_RESULTS = None  # test.py reads exec_time_ns off this


def _build():
    # Bacc (not raw Bass): its compile() runs generate_event_semaphores(),
    # which splits multi-wait sync_infos down to the 1-wait HW limit.
    nc = bacc.Bacc("TRN2", target_bir_lowering=False, debug=False, num_devices=N_CORES)

    xt = nc.dram_tensor("xt", [KP, BPC], BF16, kind="ExternalInput").ap()
    w1t = nc.dram_tensor("w1t", [KP, H], BF16, kind="ExternalInput").ap()
    w2t = nc.dram_tensor("w2t", [H, H], BF16, kind="ExternalInput").ap()
    w3t = nc.dram_tensor("w3t", [H, OUT], BF16, kind="ExternalInput").ap()
    g1 = nc.dram_tensor("g1", [128, HO], F32, kind="ExternalInput").ap()
    g2 = nc.dram_tensor("g2", [128, HO], F32, kind="ExternalInput").ap()
    outt = nc.dram_tensor("outt", [OUT, BPC], F32, kind="ExternalOutput").ap()

    with tile.TileContext(nc) as tc, ExitStack() as ctx:
        wp = ctx.enter_context(tc.tile_pool(name="weights", bufs=1))
        xp = ctx.enter_context(tc.tile_pool(name="x", bufs=3))
        hp = ctx.enter_context(tc.tile_pool(name="h", bufs=2))
        rp = ctx.enter_context(tc.tile_pool(name="rsq", bufs=2))
        op = ctx.enter_context(tc.tile_pool(name="out", bufs=3))
        pp = ctx.enter_context(tc.tile_pool(name="ps", bufs=3, space="PSUM"))
        sp = ctx.enter_context(tc.tile_pool(name="ssq", bufs=2, space="PSUM"))
        p3 = ctx.enter_context(tc.tile_pool(name="ps3", bufs=2, space="PSUM"))

        # --- resident weights -------------------------------------------------
        w1sb = wp.tile([128, K1, H], BF16)
        for k in range(K1):
            nc.sync.dma_start(w1sb[:, k, :], w1t[k * 128 : (k + 1) * 128, :])
        w2sb = wp.tile([128, K2, H], BF16)
        for k in range(K2):
            nc.sync.dma_start(w2sb[:, k, :], w2t[k * 128 : (k + 1) * 128, :])
        w3sb = wp.tile([128, K2, OUT], BF16)
        for k in range(K2):
            nc.sync.dma_start(w3sb[:, k, :], w3t[k * 128 : (k + 1) * 128, :])
        g1sb = wp.tile([128, HO], F32)
        nc.sync.dma_start(g1sb[:], g1[:])
        g2sb = wp.tile([128, HO], F32)
        nc.sync.dma_start(g2sb[:], g2[:])
        ones = wp.tile([128, 128], BF16)
        nc.vector.memset(ones[:], 1.0 / H)

        xt_r = xt.rearrange("(k p) b -> p k b", p=128)

        def layer(rhs, w_sb, nk, g_sb, sq_on_act):
            """rhs [128, nk, BS] bf16 -> gelu(rmsnorm(rhs.T @ W)) [128, HO, BS] bf16."""
            hraw = hp.tile([128, HO, BS], BF16, tag="hraw")
            hsq = hp.tile([128, HO, BS], BF16, tag="hsq")
            hs = hp.tile([128, HO, BS], BF16, tag="hs")
            hout = hp.tile([128, HO, BS], BF16, tag="hout")
            ssq = sp.tile([128, BS], F32, tag="ssq")
            for oi in range(HO):
                ps = pp.tile([128, BS], F32, tag="mm")
                for k in range(nk):
                    nc.tensor.matmul(
                        ps[:],
                        lhsT=w_sb[:, k, oi * 128 : (oi + 1) * 128],
                        rhs=rhs[:, k, :],
                        start=(k == 0),
                        stop=(k == nk - 1),
                    )
                nc.scalar.copy(hraw[:, oi, :], ps[:])
                if sq_on_act:
                    nc.scalar.square(hsq[:, oi, :], ps[:])
                else:
                    # PSUM allows only one DVE read port -> square the SBUF copy
                    nc.vector.tensor_mul(hsq[:, oi, :], hraw[:, oi, :], hraw[:, oi, :])
            # mean over all H features, replicated across 128 partitions
            for oi in range(HO):
                nc.tensor.matmul(
                    ssq[:], lhsT=ones[:], rhs=hsq[:, oi, :],
                    start=(oi == 0), stop=(oi == HO - 1),
                )
            # rinv = rsqrt(mean): magic seed + 2 Newton steps (f32 on DVE)
            ti = rp.tile([128, BS], I32, tag="ti")
            nc.vector.tensor_scalar(
                ti[:], ssq[:].bitcast(I32), 1, -1,
                op0=ALU.arith_shift_right, op1=ALU.bitwise_xor,
            )  # ~(v >> 1)
            y0 = rp.tile([128, BS], I32, tag="y0")
            nc.vector.tensor_scalar(y0[:], ti[:], MAGIC + 1, None, op0=ALU.add)
            y0f = y0[:].bitcast(F32)
            t1 = rp.tile([128, BS], F32, tag="t1")
            nc.vector.tensor_mul(t1[:], y0f, y0f)
            t2 = rp.tile([128, BS], F32, tag="t2")
            nc.vector.tensor_mul(t2[:], t1[:], ssq[:])
            nc.vector.tensor_scalar(t2[:], t2[:], -0.5, 1.5, op0=ALU.mult, op1=ALU.add)
            y1 = rp.tile([128, BS], F32, tag="y1")
            nc.vector.tensor_mul(y1[:], y0f, t2[:])
            # second Newton step
            nc.vector.tensor_mul(t1[:], y1[:], y1[:])
            nc.vector.tensor_mul(t2[:], t1[:], ssq[:])
            nc.vector.tensor_scalar(t2[:], t2[:], -0.5, 1.5, op0=ALU.mult, op1=ALU.add)
            rinv = rp.tile([128, BS], BF16, tag="rinv")
            nc.vector.tensor_mul(rinv[:], y1[:], t2[:])
            for oi in range(HO):
                nc.vector.tensor_mul(hs[:, oi, :], hraw[:, oi, :], rinv[:])
                nc.scalar.activation(
                    hout[:, oi, :], hs[:, oi, :], ACTF.Gelu,
                    scale=g_sb[:, oi : oi + 1],
                )
            return hout

        for c in range(NB):
            bsl = slice(c * BS, (c + 1) * BS)
            xsb = xp.tile([128, K1, BS], BF16, tag="xsb")
            for k in range(K1):
                # SWDGE: HWDGE direct2d supports only one sync-wait slot and
                # these loads carry a WAR wait + a queue wait
                nc.gpsimd.dma_start(xsb[:, k, :], xt_r[:, k, bsl])

            h1 = layer(xsb, w1sb, K1, g1sb, sq_on_act=True)
            h2 = layer(h1, w2sb, K2, g2sb, sq_on_act=False)

            ps3 = p3.tile([OUT, BS], F32, tag="mm3")
            for k in range(K2):
                nc.tensor.matmul(
                    ps3[:], lhsT=w3sb[:, k, :], rhs=h2[:, k, :],
                    start=(k == 0), stop=(k == K2 - 1),
                )
            osb = op.tile([OUT, BS], F32, tag="osb")
            nc.scalar.copy(osb[:], ps3[:])
            nc.gpsimd.dma_start(outt[:, bsl], osb[:])

    nc.compile()
    return nc


def _quant(w):
    s = max(float(np.mean(np.abs(w))), EPS_Q)
    return np.clip(np.round(w / s), -1.0, 1.0)


def kernel(x, w1, g1, w2, g2, w3):
    global LAST_RESULTS
    bf = ml_dtypes.bfloat16

    w1q = _quant(np.asarray(w1, np.float32))  # [H, IN]
    w2q = _quant(np.asarray(w2, np.float32))  # [H, H]
    w3q = _quant(np.asarray(w3, np.float32))  # [OUT, H]

    w1t_np = np.zeros([KP, H], dtype=bf)
    w1t_np[:IN] = w1q.T.astype(bf)
    w2t_np = np.ascontiguousarray(w2q.T.astype(bf))
    w3t_np = np.ascontiguousarray(w3q.T.astype(bf))
    g1_np = np.ascontiguousarray(np.asarray(g1, np.float32).reshape(HO, 128).T)
    g2_np = np.ascontiguousarray(np.asarray(g2, np.float32).reshape(HO, 128).T)

    xt_np = np.zeros([KP, B], dtype=bf)
    xt_np[:IN] = np.asarray(x, np.float32).T.astype(bf)

    if "nc" not in _cache:
        _cache["nc"] = _build()
    nc = _cache["nc"]

    in_maps = []
    for i in range(N_CORES):
        in_maps.append(
            {
                "xt": np.ascontiguousarray(xt_np[:, i * BPC : (i + 1) * BPC]),
                "w1t": w1t_np,
                "w2t": w2t_np,
                "w3t": w3t_np,
                "g1": g1_np,
                "g2": g2_np,
            }
        )

    res = run_bass_kernel_spmd(nc, in_maps, core_ids=list(range(N_CORES)))
    LAST_RESULTS = res

    out = np.empty([B, OUT], dtype=np.float32)
    for i in range(N_CORES):
        out[i * BPC : (i + 1) * BPC] = res.results[i]["outt"].T
    return out


# revision 45
# speedup vs baseline: 157.1508x; 157.1508x over previous
"""BitNet MNIST MLP forward on 8 Trainium2 NeuronCores (pure data parallel).

Reference math (per _bitlinear): out = (x/sx) @ w_q.T * sx with per-row
sx = max(|x|) -- the activation scale cancels exactly, so we compute
x @ w_q.T directly.  Ternary w_q is precomputed on host (exact in bf16).

Per-core dataflow (batch shard 8192 rows, chunks of 512 batch columns):
  activations live feature-major [feat_part(128) x batch_free] in SBUF, so
  every layer's matmul contracts features on partitions with stationary
  (pre-transposed) weights and NO on-chip transposes.
  RMS mean(h^2) over the 1024 features = ones(1/1024)-matmul accumulated
  over the 8 feature tiles -> replicated [128, 512] PSUM value.
  rsqrt = int bit-trick seed + 2 Newton iterations on DVE (no ACT table
  thrash; ScalarE only runs {copy, square, gelu} = one table set).
  gelu(h*rinv*g): rinv via DVE tensor-tensor, g via per-partition ACT scale.
"""

import os
from contextlib import ExitStack

import numpy as np
import ml_dtypes

import concourse.bacc as bacc
import concourse.bass as bass
import concourse.mybir as mybir
import concourse.tile as tile
from concourse.bass_utils import run_bass_kernel_spmd

N_CORES = 8
B, IN, H, OUT = 65536, 784, 1024, 10
BPC = B // N_CORES  # 8192 rows per core
KP = 896            # 784 zero-padded to 7*128
K1 = KP // 128      # 7 contraction tiles, layer 1
K2 = H // 128       # 8 contraction tiles, layers 2/3
HO = H // 128       # 8 output-feature tiles
BS = 512            # batch columns per chunk
NB = BPC // BS      # 16 chunks
EPS_Q = 1e-5
MAGIC = 0x5F3759DF

F32 = mybir.dt.float32
BF16 = mybir.dt.bfloat16
I32 = mybir.dt.int32
ALU = mybir.AluOpType
ACTF = mybir.ActivationFunctionType

_cache = {}
LAST_RESULTS = None  # test.py reads exec_time_ns off this


def _build(g_is_one=True):
    # Bacc (not raw Bass): its compile() runs generate_event_semaphores(),
    # which splits multi-wait sync_infos down to the 1-wait HW limit.
    nc = bacc.Bacc("TRN2", target_bir_lowering=False, debug=False, num_devices=N_CORES)

    xt = nc.dram_tensor("xt", [KP, BPC], BF16, kind="ExternalInput").ap()
    w1t = nc.dram_tensor("w1t", [KP, H], BF16, kind="ExternalInput").ap()
    w2t = nc.dram_tensor("w2t", [H, H], BF16, kind="ExternalInput").ap()
    w3t = nc.dram_tensor("w3t", [H, OUT], BF16, kind="ExternalInput").ap()
    g1 = nc.dram_tensor("g1", [128, HO], F32, kind="ExternalInput").ap()
    g2 = nc.dram_tensor("g2", [128, HO], F32, kind="ExternalInput").ap()
    outt = nc.dram_tensor("outt", [OUT, BPC], F32, kind="ExternalOutput").ap()

    with tile.TileContext(nc) as tc, ExitStack() as ctx:
        wp = ctx.enter_context(tc.tile_pool(name="weights", bufs=1))
        xp = ctx.enter_context(tc.tile_pool(name="x", bufs=4))
        hp = ctx.enter_context(tc.tile_pool(name="h", bufs=3))
        # gelu outputs cross one pipeline stage (written superstep s, read s+1)
        hq = ctx.enter_context(tc.tile_pool(name="hout", bufs=4))
        rp = ctx.enter_context(tc.tile_pool(name="rsq", bufs=2))
        op = ctx.enter_context(tc.tile_pool(name="out", bufs=3))
        pp = ctx.enter_context(tc.tile_pool(name="ps", bufs=4, space="PSUM"))
        sp = ctx.enter_context(tc.tile_pool(name="ssq", bufs=2, space="PSUM"))
        p3 = ctx.enter_context(tc.tile_pool(name="ps3", bufs=2, space="PSUM"))

        # --- resident weights (layer-1 set first so chunk 0 starts sooner) ---
        w1sb = wp.tile([128, K1, H], BF16)
        for k in range(K1):
            nc.sync.dma_start(w1sb[:, k, :], w1t[k * 128 : (k + 1) * 128, :])
        g1sb = wp.tile([128, HO], F32)
        nc.sync.dma_start(g1sb[:], g1[:])
        ones = wp.tile([128, 128], BF16)
        nc.vector.memset(ones[:], 1.0 / H)
        # layer-2/3 weights aren't needed until superstep 1 -> emit after so
        # the HWDGE lanes serve x-chunk-0 + w1 first
        w2sb = wp.tile([128, K2, H], BF16)
        w3sb = wp.tile([128, K2, OUT], BF16)
        g2sb = wp.tile([128, HO], F32)

        def load_l23_weights():
            for k in range(K2):
                nc.sync.dma_start(w2sb[:, k, :], w2t[k * 128 : (k + 1) * 128, :])
            for k in range(K2):
                nc.sync.dma_start(w3sb[:, k, :], w3t[k * 128 : (k + 1) * 128, :])
            nc.sync.dma_start(g2sb[:], g2[:])

        xt_r = xt.rearrange("(k p) b -> p k b", p=128)

        def layer_mm(rhs, w_sb, nk, fine=False):
            """Matmul phase: h = rhs.T @ W -> hraw [128, HO, BS] bf16, plus
            squared-sum reduce tree down to octs [128, BS] (big DVE ops)."""
            hraw = hp.tile([128, HO, BS], BF16, tag="hraw")
            hsq = hp.tile([128, HO, BS], BF16, tag="hsq")
            pairs = hp.tile([128, HO // 2, BS], BF16, tag="hsqp")
            quads = hp.tile([128, 2, BS], BF16, tag="hsqq")
            octs = hp.tile([128, BS], BF16, tag="hsqo")
            for oi in range(HO):
                ps = pp.tile([128, BS], F32, tag="mm")
                for k in range(nk):
                    nc.tensor.matmul(
                        ps[:],
                        lhsT=w_sb[:, k, oi * 128 : (oi + 1) * 128],
                        rhs=rhs[:, k, :],
                        start=(k == 0),
                        stop=(k == nk - 1),
                    )
                nc.scalar.copy(hraw[:, oi, :], ps[:])
                if fine:
                    nc.vector.tensor_mul(hsq[:, oi, :], hraw[:, oi, :], hraw[:, oi, :])
                    if oi % 2 == 1:
                        nc.vector.tensor_add(
                            pairs[:, oi // 2, :], hsq[:, oi - 1, :], hsq[:, oi, :]
                        )
            if not fine:
                nc.vector.tensor_mul(hsq[:], hraw[:], hraw[:])
                ev = hsq[:].rearrange("p (j two) f -> p two j f", two=2)
                nc.vector.tensor_add(pairs[:], ev[:, 0], ev[:, 1])
            nc.vector.tensor_add(quads[:], pairs[:, 0:2, :], pairs[:, 2:4, :])
            nc.vector.tensor_add(octs[:], quads[:, 0, :], quads[:, 1, :])
            return hraw, octs

        def layer_norm(state, g_sb, g_is_one, fine=False):
            """Norm phase: ONE ones-matmul partition reduce, rsqrt via magic
            seed + 1 Newton step, prescale, gelu -> hout [128, HO, BS]."""
            hraw, octs = state
            hs = hp.tile([128, HO, BS], BF16, tag="hs")
            hout = hq.tile([128, HO, BS], BF16, tag="hout")
            ssq = sp.tile([128, BS], F32, tag="ssq")
            nc.tensor.matmul(ssq[:], lhsT=ones[:], rhs=octs[:], start=True, stop=True)
            ti = rp.tile([128, BS], I32, tag="ti")
            nc.vector.tensor_scalar(
                ti[:], ssq[:].bitcast(I32), 1, -1,
                op0=ALU.arith_shift_right, op1=ALU.bitwise_xor,
            )  # ~(v >> 1)
            y0 = rp.tile([128, BS], I32, tag="y0")
            nc.vector.tensor_scalar(y0[:], ti[:], MAGIC + 1, None, op0=ALU.add)
            y0f = y0[:].bitcast(F32)
            t1 = rp.tile([128, BS], F32, tag="t1")
            nc.vector.tensor_mul(t1[:], y0f, y0f)
            t2 = rp.tile([128, BS], F32, tag="t2")
            nc.vector.tensor_mul(t2[:], t1[:], ssq[:])
            nc.vector.tensor_scalar(t2[:], t2[:], -0.5, 1.5, op0=ALU.mult, op1=ALU.add)
            rinv = rp.tile([128, BS], BF16, tag="rinv")
            nc.vector.tensor_mul(rinv[:], y0f, t2[:])
            rb = rinv[:].rearrange("p (o f) -> p o f", o=1).broadcast_to([128, HO, BS])
            if fine:
                for oi in range(HO):
                    nc.vector.tensor_mul(hs[:, oi, :], hraw[:, oi, :], rinv[:])
                    if g_is_one:
                        nc.scalar.activation(hout[:, oi, :], hs[:, oi, :], ACTF.Gelu)
                    else:
                        nc.scalar.activation(
                            hout[:, oi, :], hs[:, oi, :], ACTF.Gelu,
                            scale=g_sb[:, oi : oi + 1],
                        )
            elif g_is_one:
                nc.vector.tensor_mul(hs[:], hraw[:], rb)
                nc.scalar.activation(hout[:], hs[:], ACTF.Gelu)
            else:
                nc.vector.tensor_mul(hs[:], hraw[:], rb)
                for oi in range(HO):
                    nc.scalar.activation(
                        hout[:, oi, :], hs[:, oi, :], ACTF.Gelu,
                        scale=g_sb[:, oi : oi + 1],
                    )
            return hout

        # 3-stage software pipeline over chunks: in superstep s emit L1(s),
        # L2(s-1), L3(s-2).  This puts chunk s's L1 matmuls ahead of chunk
        # s-1's L2 in PE program order, so PE has ready work while the
        # norm/gelu chain of the previous stage runs on DVE/ACT.
        h1s: dict[int, object] = {}
        h2s: dict[int, object] = {}
        for s in range(NB + 2):
            if s < NB:
                bsl = slice(s * BS, (s + 1) * BS)
                xsb = xp.tile([128, K1, BS], BF16, tag="xsb")
                for k in range(K1):
                    nc.sync.dma_start(xsb[:, k, :], xt_r[:, k, bsl])
                h1s[s] = layer_norm(layer_mm(xsb, w1sb, K1), g1sb, g_is_one)
            if s == 0:
                load_l23_weights()
            if 1 <= s <= NB:
                fine = s == NB  # final chunk: minimize norm->L3 latency
                h2s[s - 1] = layer_norm(
                    layer_mm(h1s.pop(s - 1), w2sb, K2, fine=fine),
                    g2sb, g_is_one, fine=fine,
                )
            if s >= 2:
                c = s - 2
                h2 = h2s.pop(c)
                # L3 (M=10): pack 4 col-strips of the PE array concurrently,
                # 2 K-chunks accumulated per strip; strips merge on ACT+DVE.
                ps3 = p3.tile([128, BS], F32, tag="mm3")
                for g in range(4):
                    for kk in range(2):
                        k = 2 * g + kk
                        nc.tensor.matmul(
                            ps3[32 * g : 32 * g + OUT, :],
                            lhsT=w3sb[:, k, :],
                            rhs=h2[:, k, :],
                            start=(kk == 0),
                            stop=(kk == 1),
                            tile_position=(0, 32 * g),
                        )
                osb = op.tile([OUT, BS], F32, tag="osb")
                nc.scalar.copy(osb[:], ps3[0:OUT, :])
                for g in range(1, 4):
                    nc.vector.tensor_add(osb[:], osb[:], ps3[32 * g : 32 * g + OUT, :])
                nc.sync.dma_start(outt[:, c * BS : (c + 1) * BS], osb[:])

    nc.compile()
    return nc


def _quant(w):
    s = max(float(np.mean(np.abs(w))), EPS_Q)
    return np.clip(np.round(w / s), -1.0, 1.0)


def kernel(x, w1, g1, w2, g2, w3):
    global LAST_RESULTS
    bf = ml_dtypes.bfloat16

    w1q = _quant(np.asarray(w1, np.float32))  # [H, IN]
    w2q = _quant(np.asarray(w2, np.float32))  # [H, H]
    w3q = _quant(np.asarray(w3, np.float32))  # [OUT, H]

    w1t_np = np.zeros([KP, H], dtype=bf)
    w1t_np[:IN] = w1q.T.astype(bf)
    w2t_np = np.ascontiguousarray(w2q.T.astype(bf))
    w3t_np = np.ascontiguousarray(w3q.T.astype(bf))
    g1_np = np.ascontiguousarray(np.asarray(g1, np.float32).reshape(HO, 128).T)
    g2_np = np.ascontiguousarray(np.asarray(g2, np.float32).reshape(HO, 128).T)

    xt_np = np.zeros([KP, B], dtype=bf)
    xt_np[:IN] = np.asarray(x, np.float32).T.astype(bf)

    g_is_one = bool(np.all(np.asarray(g1) == 1.0) and np.all(np.asarray(g2) == 1.0))
    key = ("nc", g_is_one)
    if key not in _cache:
        _cache[key] = _build(g_is_one)
    nc = _cache[key]

    in_maps = []
    for i in range(N_CORES):
        in_maps.append(
            {
                "xt": np.ascontiguousarray(xt_np[:, i * BPC : (i + 1) * BPC]),
                "w1t": w1t_np,
                "w2t": w2t_np,
                "w3t": w3t_np,
                "g1": g1_np,
                "g2": g2_np,
            }
        )

    res = run_bass_kernel_spmd(nc, in_maps, core_ids=list(range(N_CORES)))
    LAST_RESULTS = res

    out = np.empty([B, OUT], dtype=np.float32)
    for i in range(N_CORES):
        out[i * BPC : (i + 1) * BPC] = res.results[i]["outt"].T
    return out
